# revision 3
# baseline (speedup 1.0000x reference)
"""CHGCNN hypergraph-conv forward on 8 Trainium2 NeuronCores (Bass/Tile).

Strategy (per core, SPMD single NEFF):
  - Edges sharded across 8 cores (dealt round-robin by degree). Nodes canonical 0..N-1.
  - Layer l: T table [N,64] (h_l) in each core's HBM (replicated).
    P1: e_feat_piece[(e,src_block)] = sum of T[node] over pairs (gather chunks via
        dma_gather + per-slot scale + strided DVE window reduce + dma_scatter_add of
        unique piece rows into per-(srcblk, dsthalf) regions).
    L2-P1: e_feat = sum of 4 piece regions (plain DMA + strided reduce, regions pre-zeroed).
    P2: partial[(n, e_half)] pieces from gathers of e_feat (scale = Binv*Dinv per pair),
        scattered into per-(e_half, n_quarter) regions.
    L2-P2: ARIN[n] = sum of 2 pieces  (canonical order, sequential).
    AllReduce(ARIN) -> ARO (full raw2 = Dinv*A*Binv*A^T h, all cores).
    Dense: stats via augmented matmul C=[h|1]^T[h|1]; BN folded into W'=W*diag(a),
    c'=a*b+beta-a*mean_y; apply: h_{l+1} = softplus(raw2 @ W' + c') via PE-transpose+matmul.
  - Pooling: per-graph mean via small matmuls; FC stack; out [512,1] (identical on cores).
"""
import sys, os
sys.path.insert(0, "/opt/trn_rl_repo")
import numpy as np

class _KBDone(Exception):
    pass

LAST_EXEC_NS = None

P = 128
NCORES = 8
MAXNI = 1024          # validated dma_gather/scatter max rows per instruction
MAXCOLS = MAXNI // P  # 8 columns per chunk
BLK = 25000           # node table block size for int16 gathers (<32768)

# ----------------------------------------------------------------------------
# Host schedule construction
# ----------------------------------------------------------------------------

def _wrap16(v):
    """dma_gather/scatter idx layout: slot i -> [i%16, i//16], replicated to 128 rows."""
    n = len(v)
    assert n % 16 == 0
    w = np.asarray(v, np.int16).reshape(n // 16, 16).T
    return np.tile(w, (8, 1))  # [128, n//16]


class AggSched:
    """Uniform (cross-core) schedule for one gather->reduce->scatter pass.

    Per core data streams: idx (int16 wrapped), scale (fp32 [128, cols]),
    sidx (int16 wrapped scatter rows).
    chunks: list of (src_row_off, n_slots, idx_coloff, scale_coloff,
                     runs=[(col0, nw, D)], region, n_rows, sidx_off, serial)
    """
    def __init__(self):
        self.chunks = []
        self.idx = [[] for _ in range(NCORES)]     # per-core list of [128, k] blocks
        self.scale = [[] for _ in range(NCORES)]   # per-core list of [128, c] blocks
        self.sidx = [[] for _ in range(NCORES)]
        self.idx_cols = 0
        self.scale_cols = 0
        self.sidx_cols = 0


def build_agg(per_core_segs, src_block_of, src_off_of, n_regions, region_rows):
    """per_core_segs: dict key -> list over cores of lists of
         (dst_region, dst_row, [(src_row_global, scale), ...])
       key must include (region, src_block); all segs of a key share both.
       src_block_of/src_off_of: key -> src block row offset (for gather in_ap)
       Region dump row = region_rows[r] (scatter pad target).
    Returns AggSched. Segments are padded across cores for SPMD uniformity.
    """
    s = AggSched()
    for key in sorted(per_core_segs.keys()):
        percore = per_core_segs[key]
        region = None
        nseg = max(len(x) for x in percore)
        if nseg == 0:
            continue
        # sort by count desc per core
        percore = [sorted(x, key=lambda t: -len(t[2])) for x in percore]
        src_off = src_off_of(key)
        # windows of 128 segments
        nwin = (nseg + P - 1) // P
        w0 = 0
        pend_windows = []  # (D, per-core [(dst_row, srcs, scales)] x128)
        for w in range(nwin):
            lo = w * P
            D = 0
            for c in range(NCORES):
                if lo < len(percore[c]):
                    D = max(D, len(percore[c][lo][2]))
            D = max(D, 1)
            win = []
            for c in range(NCORES):
                rows = []
                for p in range(P):
                    i = lo + p
                    if i < len(percore[c]):
                        reg, dst, pairs = percore[c][i]
                        rows.append((dst, pairs))
                    else:
                        rows.append((None, []))
                win.append(rows)
            region = key[0]
            pend_windows.append((D, win))
        # chunk windows: sum of D <= MAXCOLS
        i = 0
        while i < len(pend_windows):
            j = i
            tot = 0
            while j < len(pend_windows) and tot + pend_windows[j][0] <= MAXCOLS:
                tot += pend_windows[j][0]
                j += 1
            group = pend_windows[i:j]
            i = j
            # emit chunk
            cols = sum(d for d, _ in group)
            nw = len(group)
            idx_block = np.zeros((NCORES, cols * P), np.int64)
            sc_block = np.zeros((NCORES, P, cols), np.float32)
            sidx_block = np.zeros((NCORES, nw * P), np.int64)
            runs = []
            col0 = 0
            rr = region_rows[key[0]]
            for wi, (D, win) in enumerate(group):
                # merge equal-D consecutive into runs
                if runs and runs[-1][2] == D and runs[-1][0] + runs[-1][1] * runs[-1][2] == col0:
                    runs[-1] = (runs[-1][0], runs[-1][1] + 1, D)
                else:
                    runs.append((col0, 1, D))
                for c in range(NCORES):
                    for p in range(P):
                        dst, pairs = win[c][p]
                        sidx_block[c, wi * P + p] = rr if dst is None else dst
                        for t in range(D):
                            col = col0 + t
                            slot = col * P + p
                            if t < len(pairs):
                                sr, sv = pairs[t]
                                idx_block[c, slot] = sr - src_off
                                sc_block[c, p, col] = sv
                            else:
                                idx_block[c, slot] = 0
                                sc_block[c, p, col] = 0.0
                col0 += D
            for c in range(NCORES):
                s.idx[c].append(_wrap16(idx_block[c]))
                s.scale[c].append(sc_block[c])
                s.sidx[c].append(_wrap16(sidx_block[c]))
            s.chunks.append(dict(
                src_off=src_off, n_slots=cols * P, cols=cols,
                idx_off=s.idx_cols, scale_off=s.scale_cols,
                runs=runs, region=region, n_rows=nw * P, nw=nw,
                sidx_off=s.sidx_cols))
            s.idx_cols += cols * P // 16
            s.scale_cols += cols
            s.sidx_cols += nw * P // 16
    return s


def host_prep(x, node_idx, edge_idx, batch):
    N = x.shape[0]
    I = node_idx.shape[0]
    E = int(edge_idx.max()) + 1 if len(edge_idx) else 1
    G = int(batch.max()) + 1
    node_idx = np.asarray(node_idx, np.int64)
    edge_idx = np.asarray(edge_idx, np.int64)
    batch = np.asarray(batch, np.int64)

    deg_e = np.bincount(edge_idx, minlength=E)
    deg_n = np.bincount(node_idx, minlength=N)
    Binv = np.where(deg_e > 0, 1.0 / np.maximum(deg_e, 1), 0.0).astype(np.float32)
    Dinv = np.where(deg_n > 0, 1.0 / np.maximum(deg_n, 1), 0.0).astype(np.float32)

    # ---- edge shard: live edges dealt by degree
    live = np.nonzero(deg_e > 0)[0]
    order = live[np.argsort(-deg_e[live], kind="stable")]
    core_of_edge = np.full(E, -1, np.int64)
    epos = np.full(E, -1, np.int64)   # local e_feat row
    counts = [0] * NCORES
    for i, e in enumerate(order):
        c = i % NCORES
        core_of_edge[e] = c
        epos[e] = counts[c]
        counts[c] += 1
    RE = max(counts)
    # e_feat halves (128-aligned so EF half offsets match stream indices)
    EH = (((RE + 1) // 2 + 127) // 128) * 128
    assert EH <= 32767

    # pairs grouped per (core)
    pair_core = core_of_edge[edge_idx]

    # ---- P1 segments: key (region=(srcblk, dsthalf)), seg=(dst=piece row, srcs)
    nblk = (N + BLK - 1) // BLK
    p1_regions = {}
    for b in range(nblk):
        for h in range(2):
            p1_regions[(b, h)] = len(p1_regions)
    p1_region_size = EH + 1  # + dump row

    segs1 = {}
    src_blk = node_idx // BLK
    for c in range(NCORES):
        sel = np.nonzero(pair_core == c)[0]
        ni = node_idx[sel]
        ei = edge_idx[sel]
        bi = src_blk[sel]
        ep = epos[ei]
        hh = (ep >= EH).astype(np.int64)
        erow = ep - hh * EH
        # group by (b, h, e)
        key_arr = (bi * 2 + hh) * (EH + 1) + erow
        o = np.argsort(key_arr, kind="stable")
        ni, bi, hh, erow = ni[o], bi[o], hh[o], erow[o]
        ka = key_arr[o]
        bounds = np.nonzero(np.diff(ka))[0] + 1
        starts = np.concatenate([[0], bounds])
        ends = np.concatenate([bounds, [len(ka)]])
        for si, se in zip(starts, ends):
            b = int(bi[si]); h = int(hh[si]); r = int(erow[si])
            key = (b, h)
            reg = p1_regions[key]
            segs1.setdefault((reg, b), [[] for _ in range(NCORES)])[c].append(
                (reg, r, [(int(n), 1.0) for n in ni[si:se]]))
    # split counts > MAXCOLS
    _split_big(segs1, MAXCOLS)
    sched1 = build_agg(segs1, None, lambda k: (k[1]) * BLK, len(p1_regions),
                       {p1_regions[k]: EH for k in p1_regions})

    # ---- L2-P1: e_feat[rows RE] = sum over nblk regions at same (h, row)
    # handled densely in kernel (no schedule needed beyond sizes)

    # ---- P2 segments: dst piece (n, e_half) -> region (h, quarter(n))
    NQ = (N + 3) // 4
    assert NQ <= 32767
    p2_regions = {}
    for h in range(2):
        for q in range(4):
            p2_regions[(h, q)] = len(p2_regions)
    p2_region_size = NQ + 1

    segs2 = {}
    sc2 = Binv[edge_idx] * Dinv[node_idx]
    for c in range(NCORES):
        sel = np.nonzero(pair_core == c)[0]
        ni = node_idx[sel]
        ei = edge_idx[sel]
        sv = sc2[sel]
        ep = epos[ei]
        hh = (ep >= EH).astype(np.int64)
        erow = ep - hh * EH
        qq = ni // NQ
        nrow = ni - qq * NQ
        key_arr = ((hh * 4 + qq) * (NQ + 1) + nrow)
        o = np.argsort(key_arr, kind="stable")
        ni, sv, hh, erow, qq, nrow = ni[o], sv[o], hh[o], erow[o], qq[o], nrow[o]
        ka = key_arr[o]
        bounds = np.nonzero(np.diff(ka))[0] + 1
        starts = np.concatenate([[0], bounds])
        ends = np.concatenate([bounds, [len(ka)]])
        for si, se in zip(starts, ends):
            h = int(hh[si]); q = int(qq[si]); r = int(nrow[si])
            reg = p2_regions[(h, q)]
            segs2.setdefault((reg, h), [[] for _ in range(NCORES)])[c].append(
                (reg, r, [(int(h * EH + er), float(s)) for er, s in
                          zip(erow[si:se], sv[si:se])]))
    _split_big(segs2, MAXCOLS)
    sched2 = build_agg(segs2, None, lambda k: (k[1]) * EH, len(p2_regions),
                       {p2_regions[k]: NQ for k in p2_regions})

    # ---- pooling: per node-tile matmul pieces
    # graphs contiguous in node order (batch sorted)
    cnt_g = np.bincount(batch, minlength=G).astype(np.float32)
    pool_meta = []   # (tile, window, col_lo_graph, ncols, startflags per window handled in build)
    NT = (N + P - 1) // P
    pool_cols = []
    for t in range(NT):
        lo, hi = t * P, min((t + 1) * P, N)
        gs = batch[lo:hi]
        g0, g1 = int(gs[0]), int(gs[-1])
        # windows of 128 graphs
        w0, w1 = g0 // P, g1 // P
        for w in range(w0, w1 + 1):
            glo = max(g0, w * P)
            ghi = min(g1, (w + 1) * P - 1)
            ncol = ghi - glo + 1
            Pt = np.zeros((P, ncol), np.float32)
            for p in range(hi - lo):
                g = int(gs[p])
                if w * P <= g <= ghi and g >= glo:
                    Pt[p, g - glo] = 1.0 / max(cnt_g[g], 1.0)
            pool_meta.append(dict(tile=t, window=w, grow=glo - w * P, ncol=ncol,
                                  col_off=sum(c.shape[1] for c in pool_cols)))
            pool_cols.append(Pt)
    pool_data = np.concatenate(pool_cols, axis=1) if pool_cols else np.zeros((P, 1), np.float32)

    return dict(
        N=N, E=E, I=I, G=G, RE=RE, EH=EH, NQ=NQ, nblk=nblk,
        deg_e=deg_e, deg_n=deg_n, Binv=Binv, Dinv=Dinv,
        core_of_edge=core_of_edge, epos=epos,
        sched1=sched1, sched2=sched2,
        n_p1_regions=len(p1_regions), p1_region_size=p1_region_size,
        n_p2_regions=len(p2_regions), p2_region_size=p2_region_size,
        pool_meta=pool_meta, pool_data=pool_data, cnt_g=cnt_g,
    )


def _split_big(segs, maxc):
    # Oversized segments are split into levels; each level becomes its OWN key so
    # duplicate destinations never share a scatter instruction (CCE RMW races).
    for key in list(segs):
        base = segs.pop(key)
        levels = {}
        for c in range(NCORES):
            for reg, dst, pairs in base[c]:
                for lv, j in enumerate(range(0, len(pairs), maxc)):
                    levels.setdefault(lv, [[] for _ in range(NCORES)])[c].append(
                        (reg, dst, pairs[j:j + maxc]))
        for lv, percore in levels.items():
            segs[key + (lv,)] = percore


# ----------------------------------------------------------------------------
# numpy executor for schedule validation (mirrors device semantics)
# ----------------------------------------------------------------------------

def numpy_agg(sched, src_tables, region_tables, core):
    """src_tables: full source table [rows, F] (np); region_tables: list of zeroed np arrays."""
    F = 64
    for ch in sched.chunks:
        cols = ch["cols"]
        idx = _unwrap(np.concatenate(sched.idx[core], axis=1), ch["idx_off"], ch["n_slots"])
        sc = np.concatenate(sched.scale[core], axis=1)[:, ch["scale_off"]:ch["scale_off"] + cols]
        g = src_tables[ch["src_off"] + idx.reshape(cols, P)]  # [cols, P, F] slot=(c*128+p)
        g = g.transpose(1, 0, 2) * sc[:, :, None]             # [P, cols, F]
        stage = np.zeros((P, ch["nw"], F), np.float32)
        wi = 0
        for (c0, nw, D) in ch["runs"]:
            for k in range(nw):
                stage[:, wi] = g[:, c0 + k * D:c0 + (k + 1) * D].sum(axis=1)
                wi += 1
        sidx = _unwrap(np.concatenate(sched.sidx[core], axis=1), ch["sidx_off"], ch["n_rows"])
        rt = region_tables[ch["region"]]
        for i in range(ch["n_rows"]):
            r = sidx[i]
            rt[r] += stage[i % P, i // P]


def _unwrap(stream, off, n):
    blk = stream[:16, off:off + n // 16]
    return blk.T.reshape(-1)[:n].astype(np.int64)


# ----------------------------------------------------------------------------
# Bass kernel builder
# ----------------------------------------------------------------------------

def build_nc(pre, AD, NLAYERS=3, HD=128):
    import concourse.bass as bass
    import concourse.mybir as mybir
    from concourse import bacc
    from concourse.tile import TileContext
    from concourse.masks import make_identity

    F = 64
    AF = mybir.ActivationFunctionType
    OP = mybir.AluOpType
    f32 = mybir.dt.float32
    i16 = mybir.dt.int16

    N = pre["N"]; G = pre["G"]
    NPAD = ((N + 255) // 256) * 256
    NT = NPAD // P                      # node tiles
    NCH = NT // 2                       # apply chunks (2 tiles each)
    EH = pre["EH"]; NQ = pre["NQ"]
    EF_ROWS = ((2 * EH + 127) // 128) * 128
    R1 = ((pre["p1_region_size"] + 127) // 128) * 128
    R2 = ((pre["p2_region_size"] + 127) // 128) * 128
    NR1 = pre["n_p1_regions"]; NR2 = pre["n_p2_regions"]
    s1, s2 = pre["sched1"], pre["sched2"]
    GW = (G + P - 1) // P               # pooled windows

    nc = bacc.Bacc(num_devices=NCORES)
    # ---- inputs
    xT = nc.dram_tensor("xT", [AD, NPAD], f32, kind="ExternalInput")
    W_emb = nc.dram_tensor("W_emb", [AD, F], f32, kind="ExternalInput")
    b_emb = nc.dram_tensor("b_emb", [1, F], f32, kind="ExternalInput")
    convW2 = nc.dram_tensor("convW2", [NLAYERS * P, F], f32, kind="ExternalInput")
    conv_bc = nc.dram_tensor("conv_bc", [F, NLAYERS], f32, kind="ExternalInput")
    bn_gc = nc.dram_tensor("bn_gc", [F, NLAYERS], f32, kind="ExternalInput")
    bn_bc = nc.dram_tensor("bn_bc", [F, NLAYERS], f32, kind="ExternalInput")
    fc_W = nc.dram_tensor("fc_W", [F, HD], f32, kind="ExternalInput")
    fc_b = nc.dram_tensor("fc_b", [1, HD], f32, kind="ExternalInput")
    fco_W = nc.dram_tensor("fco_W", [HD, 1], f32, kind="ExternalInput")
    fco_b = nc.dram_tensor("fco_b", [1, 1], f32, kind="ExternalInput")
    p1_idx = nc.dram_tensor("p1_idx", [P, max(s1.idx_cols, 1)], i16, kind="ExternalInput")
    p1_sc = nc.dram_tensor("p1_sc", [P, max(s1.scale_cols, 1)], f32, kind="ExternalInput")
    p1_sx = nc.dram_tensor("p1_sx", [P, max(s1.sidx_cols, 1)], i16, kind="ExternalInput")
    p2_idx = nc.dram_tensor("p2_idx", [P, max(s2.idx_cols, 1)], i16, kind="ExternalInput")
    p2_sc = nc.dram_tensor("p2_sc", [P, max(s2.scale_cols, 1)], f32, kind="ExternalInput")
    p2_sx = nc.dram_tensor("p2_sx", [P, max(s2.sidx_cols, 1)], i16, kind="ExternalInput")
    PCOLS = pre["pool_data"].shape[1]
    pool_d = nc.dram_tensor("pool_d", [P, PCOLS], f32, kind="ExternalInput")
    # ---- internal tables
    HTAB = nc.dram_tensor("HTAB", [NPAD, F], f32)
    EF = nc.dram_tensor("EF", [EF_ROWS, F], f32)
    REG1 = [nc.dram_tensor(f"REG1_{l}_{r}", [R1, F], f32)
            for l in range(NLAYERS) for r in range(NR1)]
    REG2 = [nc.dram_tensor(f"REG2_{l}_{r}", [R2, F], f32)
            for l in range(NLAYERS) for r in range(NR2)]
    ARIN = nc.dram_tensor("ARIN", [NPAD, F], f32)
    ARO = nc.dram_tensor("ARO", [NPAD, F], f32, addr_space="Shared")
    OUT = nc.dram_tensor("OUT", [GW * P, 1], f32, kind="ExternalOutput")

    KB = int(os.environ.get("KBISECT", "9"))
    with TileContext(nc) as tc:
        with (
            tc.tile_pool(name="const", bufs=1) as cpool,
            tc.tile_pool(name="gbuf", bufs=3) as gpool,
            tc.tile_pool(name="stage", bufs=3) as spool,
            tc.tile_pool(name="small", bufs=2) as mpool,
            tc.tile_pool(name="dense", bufs=3) as dpool,
            tc.tile_pool(name="psum", bufs=4, space="PSUM") as ppool,
            tc.tile_pool(name="psA", bufs=1, space="PSUM") as psA,
        ):
            ident = cpool.tile([P, P], f32)
            make_identity(nc, ident[:])
            ones_col = cpool.tile([P, 1], f32)
            nc.gpsimd.memset(ones_col[:], 1.0)
            ones_row = cpool.tile([1, P], f32)
            nc.gpsimd.memset(ones_row[:], 1.0)
            zt = cpool.tile([P, 32, F], f32)
            nc.gpsimd.memset(zt[:], 0.0)

            # ---------- zero all regions + ARIN pad rows
            def zero_table(tab, rows):
                r3 = tab[:, :].rearrange("(k p) f -> p k f", p=P)
                K = rows // P
                for k0 in range(0, K, 32):
                    kk = min(32, K - k0)
                    nc.sync.dma_start(out=r3[:, k0:k0 + kk, :], in_=zt[:, :kk, :])
            for t in REG1:
                zero_table(t, R1)
            for t in REG2:
                zero_table(t, R2)
            zero_table(ARIN, NPAD)

            # ---------- embedding: HTAB = x @ W_emb + b_emb
            wemb_t = cpool.tile([AD, F], f32)
            nc.sync.dma_start(out=wemb_t[:], in_=W_emb[:, :])
            bemb_t = cpool.tile([1, F], f32)
            nc.sync.dma_start(out=bemb_t[:], in_=b_emb[:, :])
            # broadcast b_emb to [128, 256] via PE: ones_row^T @ bemb
            bps = ppool.tile([P, F], f32, space="PSUM", tag="ps")
            nc.tensor.matmul(out=bps[:], lhsT=ones_row[:], rhs=bemb_t[:],
                             start=True, stop=True)
            bemb4 = cpool.tile([P, 4, F], f32)
            for j in range(4):
                nc.vector.tensor_copy(out=bemb4[:, j, :], in_=bps[:])
            h3 = HTAB[:, :].rearrange("(t p) f -> p t f", p=P)
            for t0 in range(0, NT, 4):
                tt = min(4, NT - t0)
                eps_ = ppool.tile([P, 4 * F], f32, space="PSUM", tag="ps")
                for j in range(tt):
                    xc = gpool.tile([AD, P], f32)
                    nc.sync.dma_start(out=xc[:], in_=xT[:, (t0 + j) * P:(t0 + j + 1) * P])
                    nc.tensor.matmul(out=eps_[:, j * F:(j + 1) * F], lhsT=xc[:],
                                     rhs=wemb_t[:], start=True, stop=True)
                hb = spool.tile([P, 4, F], f32)
                nc.vector.tensor_tensor(out=hb[:, :tt, :],
                                        in0=eps_[:, :tt * F].rearrange("p (t f) -> p t f", f=F),
                                        in1=bemb4[:, :tt, :], op=OP.add)
                nc.sync.dma_start(out=h3[:, t0:t0 + tt, :], in_=hb[:, :tt, :])

            # ---------- layers
            ef3 = EF[:, :].rearrange("(t p) f -> p t f", p=P)
            ar3 = ARIN[:, :].rearrange("(t p) f -> p t f", p=P)
            for l in range(NLAYERS if KB >= 8 else (1 if KB >= 1 else 0)):
                # ===== P1: gather HTAB -> piece regions
                def agg_pass(sched, idx_d, sc_d, sx_d, src3, src_rows, regs, use_scale):
                    for ch in sched.chunks:
                        cols = ch["cols"]; nw = ch["nw"]
                        it = gpool.tile([P, MAXNI // 16], i16)
                        nc.sync.dma_start(
                            out=it[:, :ch["n_slots"] // 16],
                            in_=idx_d[:, ch["idx_off"]:ch["idx_off"] + ch["n_slots"] // 16])
                        g = gpool.tile([P, MAXCOLS, F], f32)
                        hi_ = min(ch["src_off"] + 32760, src_rows)
                        nc.gpsimd.dma_gather(
                            out_ap=g[:, :cols, :], in_ap=src3[ch["src_off"]:hi_, :],
                            idxs_ap=it[:, :ch["n_slots"] // 16],
                            num_idxs=ch["n_slots"], num_idxs_reg=ch["n_slots"], elem_size=F)
                        if use_scale:
                            st = gpool.tile([P, MAXCOLS], f32)
                            nc.sync.dma_start(
                                out=st[:, :cols],
                                in_=sc_d[:, ch["scale_off"]:ch["scale_off"] + cols])
                            gs = gpool.tile([P, MAXCOLS, F], f32)
                            nc.vector.tensor_tensor(
                                out=gs[:, :cols, :], in0=g[:, :cols, :],
                                in1=st[:, :cols].to_broadcast([P, cols, F]), op=OP.mult)
                        else:
                            gs = g
                        stg = spool.tile([P, MAXCOLS, F], f32)
                        wi = 0
                        for (c0, rnw, D) in ch["runs"]:
                            if D == 1:
                                nc.vector.tensor_copy(out=stg[:, wi:wi + rnw, :],
                                                      in_=gs[:, c0:c0 + rnw, :])
                            else:
                                src = gs[:, c0:c0 + rnw * D, :].rearrange(
                                    "p (w d) f -> p w f d", w=rnw, d=D)
                                nc.vector.tensor_reduce(out=stg[:, wi:wi + rnw, :], in_=src,
                                                        axis=mybir.AxisListType.X, op=OP.add)
                            wi += rnw
                        xt_ = gpool.tile([P, MAXNI // 16], i16)
                        nc.sync.dma_start(
                            out=xt_[:, :ch["n_rows"] // 16],
                            in_=sx_d[:, ch["sidx_off"]:ch["sidx_off"] + ch["n_rows"] // 16])
                        nc.gpsimd.dma_scatter_add(
                            out_ap=regs[ch["region"]][:, :],
                            in_ap=stg[:, :nw, :], idxs_ap=xt_[:, :ch["n_rows"] // 16],
                            num_idxs=ch["n_rows"], num_idxs_reg=ch["n_rows"], elem_size=F)

                agg_pass(s1, p1_idx, p1_sc, p1_sx, HTAB, NPAD, REG1[l * NR1:(l + 1) * NR1], True)
                if KB == 1: continue

                # ===== L2-P1: EF[h*EH + r] = sum_b REG1[(b,h)][r]
                nblk = pre["nblk"]
                for h in range(2):
                    rh = [REG1[l * NR1 + b * 2 + h] for b in range(nblk)]
                    KH = (EH + 127) // 128  # tiles of 128 rows (region rows >= EH)
                    for k0 in range(0, KH, 8):
                        kk = min(8, KH - k0)
                        mt = spool.tile([P, 4, 8, F], f32, tag="mt")
                        if nblk < 4:
                            nc.gpsimd.memset(mt[:], 0.0)
                        for b in range(nblk):
                            rb3 = rh[b][:, :].rearrange("(t p) f -> p t f", p=P)
                            nc.sync.dma_start(out=mt[:, b, :kk, :], in_=rb3[:, k0:k0 + kk, :])
                        red = spool.tile([P, 8, F], f32, tag="red")
                        src = mt[:, :, :kk, :].rearrange("p b w f -> p w f b")
                        nc.vector.tensor_reduce(out=red[:, :kk, :], in_=src,
                                                axis=mybir.AxisListType.X, op=OP.add)
                        # EF rows h*EH + (k*128+p): not 128-aligned for h=1 in general ->
                        # EF laid out as [2, EHP] with EHP=ceil(EH/128)*128
                        EHP = ((EH + 127) // 128) * 128
                        off = h * EHP + k0 * P
                        nc.sync.dma_start(
                            out=EF[off:off + kk * P, :].rearrange("(t p) f -> p t f", p=P),
                            in_=red[:, :kk, :])

                if KB == 2: continue
                # ===== P2: gather EF -> partial piece regions (scale = Binv*Dinv)
                agg_pass(s2, p2_idx, p2_sc, p2_sx, EF, EF_ROWS, REG2[l * NR2:(l + 1) * NR2], True)

                if KB == 3: continue
                # ===== L2-P2: ARIN[n] = sum_h REG2[(h, q)][n - q*NQ]
                for q in range(4):
                    lo = q * NQ
                    hi = min(lo + NQ, N)
                    rows = hi - lo
                    KH = (rows + 127) // 128
                    for k0 in range(0, KH, 8):
                        kk = min(8, KH - k0)
                        mt = spool.tile([P, 2, 8, F], f32, tag="mt")
                        for h in range(2):
                            rb3 = REG2[l * NR2 + h * 4 + q][:, :].rearrange(
                                "(t p) f -> p t f", p=P)
                            nc.sync.dma_start(out=mt[:, h, :kk, :], in_=rb3[:, k0:k0 + kk, :])
                        red = spool.tile([P, 8, F], f32, tag="red")
                        src = mt[:, :, :kk, :].rearrange("p b w f -> p w f b")
                        nc.vector.tensor_reduce(out=red[:, :kk, :], in_=src,
                                                axis=mybir.AxisListType.X, op=OP.add)
                        # ARIN rows lo + k0*128 ... may exceed hi on last block; host
                        # guarantees NQ % 128 == 0 except last quarter; clamp rows:
                        wlim = min(kk * P, rows - k0 * P)
                        full_w = wlim // P
                        if full_w > 0:
                            nc.sync.dma_start(
                                out=ARIN[lo + k0 * P: lo + k0 * P + full_w * P, :]
                                    .rearrange("(t p) f -> p t f", p=P),
                                in_=red[:, :full_w, :])
                        remp = wlim - full_w * P
                        if remp > 0:
                            nc.sync.dma_start(
                                out=ARIN[lo + (k0 + full_w) * P: lo + (k0 + full_w) * P + remp, :],
                                in_=red[:remp, full_w, :])

                if KB == 4: continue
                # ===== AllReduce
                nc.gpsimd.collective_compute(
                    "AllReduce", OP.add, replica_groups=[list(range(NCORES))],
                    ins=[ARIN[:, :]],
                    outs=[ARO[:, :]])

                if KB == 5: continue
                # KB>=6: dense runs
                # ===== stats: C = raw2^T raw2, S1 = raw2^T 1  (over N real rows)
                o3 = ARO[:, :].rearrange("(t p) f -> p t f", p=P)
                Cps = psA.tile([F, F], f32, space="PSUM", tag="C")
                Sps = psA.tile([F, 1], f32, space="PSUM", tag="S")
                NTF = (N + P - 1) // P      # 782 tiles; last has N - (NTF-1)*128 rows
                lastp = N - (NTF - 1) * P
                for t in range(NTF):
                    rt = dpool.tile([P, F], f32)
                    pp = P if t < NTF - 1 else lastp
                    nc.sync.dma_start(out=rt[:pp, :], in_=o3[:pp, t, :])
                    nc.tensor.matmul(out=Cps[:], lhsT=rt[:pp, :], rhs=rt[:pp, :],
                                     start=(t == 0), stop=(t == NTF - 1))
                    nc.tensor.matmul(out=Sps[:], lhsT=rt[:pp, :], rhs=ones_col[:pp, :],
                                     start=(t == 0), stop=(t == NTF - 1))

                if KB == 60: continue
                # ===== postprocess: a, cfin, Wp2, cb2
                Wl = cpool.tile([P, F], f32, tag="Wl")
                nc.sync.dma_start(out=Wl[:], in_=convW2[l * P:(l + 1) * P, :])
                bcol = mpool.tile([F, 1], f32)
                nc.sync.dma_start(out=bcol[:], in_=conv_bc[:, l:l + 1])
                gcol = mpool.tile([F, 1], f32)
                nc.sync.dma_start(out=gcol[:], in_=bn_gc[:, l:l + 1])
                btcol = mpool.tile([F, 1], f32)
                nc.sync.dma_start(out=btcol[:], in_=bn_bc[:, l:l + 1])
                Cs = mpool.tile([F, F], f32)
                nc.vector.tensor_copy(out=Cs[:], in_=Cps[:])
                Ss = mpool.tile([F, 1], f32)
                nc.vector.tensor_copy(out=Ss[:], in_=Sps[:])
                mps = ppool.tile([F, 1], f32, space="PSUM", tag="ps")
                nc.tensor.matmul(out=mps[:], lhsT=Wl[:F, :], rhs=Ss[:], start=True, stop=True)
                mrw = mpool.tile([F, 1], f32)
                nc.scalar.activation(out=mrw[:], in_=mps[:], func=AF.Copy, scale=1.0 / N)
                t1ps = ppool.tile([F, F], f32, space="PSUM", tag="ps")
                nc.tensor.matmul(out=t1ps[:], lhsT=Cs[:], rhs=Wl[:F, :], start=True, stop=True)
                wt1 = mpool.tile([F, F], f32)
                nc.vector.tensor_tensor(out=wt1[:], in0=t1ps[:], in1=Wl[:F, :], op=OP.mult)
                e2ps = ppool.tile([F, 1], f32, space="PSUM", tag="ps")
                nc.tensor.matmul(out=e2ps[:], lhsT=wt1[:], rhs=ones_col[:F, :], start=True, stop=True)
                var = mpool.tile([F, 1], f32)
                nc.scalar.activation(out=var[:], in_=e2ps[:], func=AF.Copy, scale=1.0 / N)
                msq = mpool.tile([F, 1], f32)
                nc.vector.tensor_tensor(out=msq[:], in0=mrw[:], in1=mrw[:], op=OP.mult)
                nc.vector.tensor_tensor(out=var[:], in0=var[:], in1=msq[:], op=OP.subtract)
                nc.vector.tensor_scalar_add(out=var[:], in0=var[:], scalar1=1e-5)
                lnv = mpool.tile([F, 1], f32)
                nc.scalar.activation(out=lnv[:], in_=var[:], func=AF.Ln)
                rstd = mpool.tile([F, 1], f32)
                nc.scalar.activation(out=rstd[:], in_=lnv[:], func=AF.Exp, scale=-0.5)
                a_ = mpool.tile([F, 1], f32)
                nc.vector.tensor_tensor(out=a_[:], in0=gcol[:], in1=rstd[:], op=OP.mult)
                am = mpool.tile([F, 1], f32)
                nc.vector.tensor_tensor(out=am[:], in0=a_[:], in1=mrw[:], op=OP.mult)
                cfin = mpool.tile([F, 1], f32)
                nc.vector.tensor_tensor(out=cfin[:], in0=btcol[:], in1=am[:], op=OP.subtract)
                # rows
                arps = ppool.tile([1, F], f32, space="PSUM", tag="ps")
                nc.tensor.matmul(out=arps[:], lhsT=a_[:], rhs=ident[:F, :F], start=True, stop=True)
                arow = mpool.tile([1, F], f32)
                nc.vector.tensor_copy(out=arow[:], in_=arps[:])
                crps = ppool.tile([1, F], f32, space="PSUM", tag="ps")
                nc.tensor.matmul(out=crps[:], lhsT=cfin[:], rhs=ident[:F, :F], start=True, stop=True)
                crow = mpool.tile([1, F], f32)
                nc.vector.tensor_copy(out=crow[:], in_=crps[:])
                abps = ppool.tile([P, F], f32, space="PSUM", tag="ps")
                nc.tensor.matmul(out=abps[:], lhsT=ones_row[:], rhs=arow[:], start=True, stop=True)
                Wp2 = cpool.tile([P, F], f32, tag="Wp")
                nc.vector.tensor_tensor(out=Wp2[:], in0=Wl[:], in1=abps[:], op=OP.mult)
                cbps = ppool.tile([P, F], f32, space="PSUM", tag="ps")
                nc.tensor.matmul(out=cbps[:], lhsT=ones_row[:], rhs=crow[:], start=True, stop=True)
                cb2 = cpool.tile([P, 2, F], f32, tag="cb")
                nc.vector.tensor_copy(out=cb2[:, 0, :], in_=cbps[:])
                nc.vector.tensor_copy(out=cb2[:, 1, :], in_=cbps[:])

                if KB == 61: continue
                # ===== apply: HTAB = softplus(ARO @ Wp2 + cb2); fused pooling on l==2
                if (KB not in (60, 61, 62)) and l == (NLAYERS - 1 if KB >= 8 else 0):
                    poolacc = cpool.tile([F, GW * P], f32)
                    nc.gpsimd.memset(poolacc[:], 0.0)
                    poold_t = cpool.tile([P, PCOLS], f32)
                    nc.sync.dma_start(out=poold_t[:], in_=pool_d[:, :])
                    pm_by_tile = {}
                    for m in pre["pool_meta"]:
                        pm_by_tile.setdefault(m["tile"], []).append(m)
                for c in range(NCH):
                    rc = dpool.tile([P, 2, F], f32)
                    nc.sync.dma_start(out=rc[:], in_=o3[:, 2 * c:2 * c + 2, :])
                    yps = ppool.tile([P, 2 * F], f32, space="PSUM", tag="ps")
                    for j in (0, 1):
                        trp = ppool.tile([F, P], f32, space="PSUM", tag="ps")
                        nc.tensor.transpose(out=trp[:], in_=rc[:, j, :], identity=ident[:])
                        trs = dpool.tile([F, P], f32, tag="trs")
                        nc.vector.tensor_copy(out=trs[:], in_=trp[:])
                        nc.tensor.matmul(out=yps[:, j * F:(j + 1) * F], lhsT=trs[:],
                                         rhs=Wp2[0:F, :], start=True, stop=True)
                    yb = dpool.tile([P, 2 * F], f32)
                    nc.vector.tensor_tensor(out=yb[:],
                                            in0=yps[:],
                                            in1=cb2[:].rearrange("p t f -> p (t f)"), op=OP.add)
                    ex = dpool.tile([P, 2 * F], f32)
                    nc.scalar.activation(out=ex[:], in_=yb[:], func=AF.Exp)
                    hc = dpool.tile([P, 2, F], f32)
                    nc.scalar.activation(out=hc[:].rearrange("p t f -> p (t f)"), in_=ex[:],
                                         func=AF.Ln, bias=1.0, scale=1.0)
                    nc.sync.dma_start(out=h3[:, 2 * c:2 * c + 2, :], in_=hc[:])
                    if (KB not in (60, 61, 62)) and l == (NLAYERS - 1 if KB >= 8 else 0):
                        for j in (0, 1):
                            for m in pm_by_tile.get(2 * c + j, []):
                                pps = ppool.tile([F, P], f32, space="PSUM", tag="ps")
                                nc.tensor.matmul(
                                    out=pps[:, :m["ncol"]],
                                    lhsT=hc[:, j, :],
                                    rhs=poold_t[:, m["col_off"]:m["col_off"] + m["ncol"]],
                                    start=True, stop=True)
                                go = m["window"] * P + m["grow"]
                                nc.vector.tensor_tensor(
                                    out=poolacc[:, go:go + m["ncol"]],
                                    in0=poolacc[:, go:go + m["ncol"]],
                                    in1=pps[:, :m["ncol"]], op=OP.add)

            # ---------- FC head
            if KB < 6 or KB in (60, 61, 62):
                dz = dpool.tile([P, 1], f32)
                nc.gpsimd.memset(dz[:], 0.0)
                for w in range(GW):
                    nc.sync.dma_start(out=OUT[w * P:(w + 1) * P, :], in_=dz[:])
                poolacc = None
            fcw_t = cpool.tile([F, HD], f32)
            nc.sync.dma_start(out=fcw_t[:], in_=fc_W[:, :])
            if KB < 6 or KB in (60, 61, 62):
                poolacc = None
            fcb_t = mpool.tile([1, HD], f32)
            nc.sync.dma_start(out=fcb_t[:], in_=fc_b[:, :])
            fcbps = ppool.tile([P, HD], f32, space="PSUM", tag="ps")
            nc.tensor.matmul(out=fcbps[:], lhsT=ones_row[:], rhs=fcb_t[:], start=True, stop=True)
            fcb_b = cpool.tile([P, HD], f32)
            nc.vector.tensor_copy(out=fcb_b[:], in_=fcbps[:])
            fcow_t = cpool.tile([HD, 1], f32)
            nc.sync.dma_start(out=fcow_t[:], in_=fco_W[:, :])
            fcob_t = mpool.tile([1, 1], f32)
            nc.sync.dma_start(out=fcob_t[:], in_=fco_b[:, :])
            fcobps = ppool.tile([P, 1], f32, space="PSUM", tag="ps")
            nc.tensor.matmul(out=fcobps[:], lhsT=ones_row[:], rhs=fcob_t[:], start=True, stop=True)
            fcob_b = cpool.tile([P, 1], f32)
            nc.vector.tensor_copy(out=fcob_b[:], in_=fcobps[:])
            for w in range(GW if (KB >= 6 and KB not in (60, 61, 62)) else 0):
                gts = dpool.tile([F, P], f32)
                ge_ = dpool.tile([F, P], f32)
                nc.scalar.activation(out=ge_[:], in_=poolacc[:, w * P:(w + 1) * P], func=AF.Exp)
                nc.scalar.activation(out=gts[:], in_=ge_[:], func=AF.Ln, bias=1.0, scale=1.0)
                g2ps = ppool.tile([P, HD], f32, space="PSUM", tag="ps")
                nc.tensor.matmul(out=g2ps[:], lhsT=gts[:], rhs=fcw_t[:], start=True, stop=True)
                g2b = dpool.tile([P, HD], f32)
                nc.vector.tensor_tensor(out=g2b[:], in0=g2ps[:], in1=fcb_b[:], op=OP.add)
                g2e = dpool.tile([P, HD], f32)
                nc.scalar.activation(out=g2e[:], in_=g2b[:], func=AF.Exp)
                g2 = dpool.tile([P, HD], f32)
                nc.scalar.activation(out=g2[:], in_=g2e[:], func=AF.Ln, bias=1.0, scale=1.0)
                g2tp = ppool.tile([P, P], f32, space="PSUM", tag="ps")
                nc.tensor.transpose(out=g2tp[:], in_=g2[:], identity=ident[:])
                g2ts = dpool.tile([P, P], f32)
                nc.vector.tensor_copy(out=g2ts[:], in_=g2tp[:])
                y_ps = ppool.tile([P, 1], f32, space="PSUM", tag="ps")
                nc.tensor.matmul(out=y_ps[:], lhsT=g2ts[:], rhs=fcow_t[:], start=True, stop=True)
                y_t = dpool.tile([P, 1], f32)
                nc.vector.tensor_tensor(out=y_t[:], in0=y_ps[:], in1=fcob_b[:], op=OP.add)
                nc.sync.dma_start(out=OUT[w * P:(w + 1) * P, :], in_=y_t[:])
    return nc


def _run_pjrt_timed(nc, in_maps, n_cores, repeats=5):
    """Replicates run_bass_via_pjrt's multi-core branch, but keeps the jitted
    executable and times repeat executions with device-resident inputs.
    Returns (per-core results, best_exec_ns or None)."""
    import time
    import jax
    from jax.experimental.shard_map import shard_map
    from jax.sharding import Mesh, PartitionSpec, NamedSharding
    from concourse import bass2jax, mybir
    from concourse.bass2jax import _bass_exec_p, partition_id_tensor, install_neuronx_cc_hook

    install_neuronx_cc_hook()
    in_names, out_names, out_avals, zero_outs = [], [], [], []
    for alloc in nc.m.functions[0].allocations:
        if not isinstance(alloc, mybir.MemoryLocationSet):
            continue
        name = alloc.memorylocations[0].name
        partition_name = nc.partition_id_tensor.name if nc.partition_id_tensor else None
        if alloc.kind == "ExternalInput":
            if name != partition_name:
                in_names.append(name)
        elif alloc.kind == "ExternalOutput":
            out_names.append(name)
            shape = tuple(alloc.tensor_shape)
            dtype = mybir.dt.np(alloc.dtype)
            out_avals.append(jax.core.ShapedArray(shape, dtype))
            zero_outs.append(np.zeros(shape, dtype))
    n_params = len(in_names)
    n_outs = len(out_avals)
    in_names.extend(out_names)
    partition_name = nc.partition_id_tensor.name if nc.partition_id_tensor else None
    if partition_name is not None:
        in_names.append(partition_name)
    donate = tuple(range(n_params, n_params + n_outs))

    def _body(*args):
        operands = list(args)
        if partition_name is not None:
            operands.append(partition_id_tensor())
        outs = _bass_exec_p.bind(
            *operands,
            out_avals=tuple(out_avals),
            in_names=tuple(in_names),
            out_names=tuple(out_names),
            lowering_input_output_aliases=(),
            sim_require_finite=True,
            sim_require_nnan=True,
            nc=nc,
        )
        return tuple(outs)

    devices = jax.devices()[:n_cores]
    mesh = Mesh(np.asarray(devices), ("core",))
    in_specs = (PartitionSpec("core"),) * (n_params + n_outs)
    out_specs = (PartitionSpec("core"),) * len(out_names)
    sharded = jax.jit(
        shard_map(_body, mesh=mesh, in_specs=in_specs, out_specs=out_specs,
                  check_rep=False),
        donate_argnums=donate, keep_unused=True)
    sh = NamedSharding(mesh, PartitionSpec("core"))
    concat_in = [
        jax.device_put(
            np.concatenate([np.asarray(in_maps[c][name]) for c in range(n_cores)], axis=0), sh)
        for name in in_names[:n_params]
    ]
    def zeros_dev():
        return [jax.device_put(np.zeros((n_cores * z.shape[0], *z.shape[1:]), z.dtype), sh)
                for z in zero_outs]
    out_arrs = sharded(*concat_in, *zeros_dev())
    jax.block_until_ready(out_arrs)
    results = [
        {name: np.asarray(out_arrs[i]).reshape(n_cores, *out_avals[i].shape)[c]
         for i, name in enumerate(out_names)}
        for c in range(n_cores)
    ]
    best = None
    for _ in range(repeats):
        zo = zeros_dev()
        jax.block_until_ready(zo)
        t0 = time.perf_counter()
        o = sharded(*concat_in, *zo)
        jax.block_until_ready(o)
        dt = time.perf_counter() - t0
        best = dt if best is None or dt < best else best
    return results, (int(best * 1e9) if best is not None else None)


def _streams(pre, core):
    s1, s2 = pre["sched1"], pre["sched2"]
    def cat(lst, dtype, n):
        if not lst:
            return np.zeros((P, 1), dtype)
        a = np.concatenate(lst, axis=1).astype(dtype)
        assert a.shape[1] == n, (a.shape, n)
        return a
    return dict(
        p1_idx=cat(s1.idx[core], np.int16, s1.idx_cols),
        p1_sc=cat(s1.scale[core], np.float32, s1.scale_cols),
        p1_sx=cat(s1.sidx[core], np.int16, s1.sidx_cols),
        p2_idx=cat(s2.idx[core], np.int16, s2.idx_cols),
        p2_sc=cat(s2.scale[core], np.float32, s2.scale_cols),
        p2_sx=cat(s2.sidx[core], np.int16, s2.sidx_cols),
    )


def kernel(x, W_emb, b_emb, conv_W, conv_b, bn_gamma, bn_beta,
           fc_W, fc_b, fco_W, fco_b, node_idx, edge_idx, batch, use_sim=False):
    from concourse.bass_utils import run_bass_kernel_spmd
    x = np.asarray(x, np.float32)
    N, AD = x.shape
    G = int(np.asarray(batch).max()) + 1
    pre = host_prep(x, np.asarray(node_idx, np.int64), np.asarray(edge_idx, np.int64),
                    np.asarray(batch, np.int64))
    NL = np.asarray(conv_W).shape[0]
    nc = build_nc(pre, AD, NLAYERS=NL, HD=np.asarray(fc_W).shape[1])
    nc.finalize()
    NPAD = ((N + 255) // 256) * 256
    xTp = np.zeros((AD, NPAD), np.float32)
    xTp[:, :N] = x.T
    convW2 = np.concatenate([np.concatenate([w, w], axis=0) for w in np.asarray(conv_W, np.float32)], axis=0)
    common = dict(
        xT=xTp, W_emb=np.asarray(W_emb, np.float32),
        b_emb=np.asarray(b_emb, np.float32).reshape(1, -1),
        convW2=convW2,
        conv_bc=np.asarray(conv_b, np.float32).T.copy(),
        bn_gc=np.asarray(bn_gamma, np.float32).T.copy(),
        bn_bc=np.asarray(bn_beta, np.float32).T.copy(),
        fc_W=np.asarray(fc_W, np.float32),
        fc_b=np.asarray(fc_b, np.float32).reshape(1, -1),
        fco_W=np.asarray(fco_W, np.float32),
        fco_b=np.asarray(fco_b, np.float32).reshape(1, 1),
        pool_d=pre["pool_data"].astype(np.float32),
    )
    in_maps = []
    for c in range(NCORES):
        m = dict(common)
        m.update(_streams(pre, c))
        in_maps.append(m)
    if use_sim:
        from concourse.bass_interp import MultiCoreSim
        sim = MultiCoreSim(nc, num_cores=NCORES, num_workers=8)
        for cid, cs in sim.cores.items():
            for name, arr in in_maps[cid].items():
                cs.tensor(name)[:] = arr
        sim.simulate()
        out = np.array(sim.cores[0].tensor("OUT"))[:G]
        return out.astype(np.float32)
    global LAST_EXEC_NS
    if bool(int(os.environ.get("KBENCH", "0"))):
        results, best_ns = _run_pjrt_timed(
            nc, in_maps, NCORES, repeats=int(os.environ.get("KREPEATS", "5")))
        LAST_EXEC_NS = best_ns
        out = results[0]["OUT"][:G]
        return out.astype(np.float32)
    trace = bool(int(os.environ.get("KTRACE", "0")))
    res = run_bass_kernel_spmd(nc, in_maps, core_ids=list(range(NCORES)), trace=trace)
    LAST_EXEC_NS = res.exec_time_ns
    out = res.results[0]["OUT"][:G]
    return out.astype(np.float32)



# revision 5
# speedup vs baseline: 5.7576x; 5.7576x over previous
"""CHGCNN hypergraph-conv forward on 8 Trainium2 NeuronCores — v2.

Per-core strategy (SPMD single NEFF):
  - Node table HTAB_P: fp16, 4-node-packed rows [Npad/4, 256] (512B rows) so a
    single dma_gather window (int16 idx) covers all nodes. Per-slot scale
    stream carries lane-mask x Binv, so gather+mask-mult+windowed-reduce
    computes e_feat with no scatter: edges are rank-sorted by degree,
    epos = p*W + w, and reduce output writes EF_P contiguously (p-major).
  - EF_P: fp16 4-edge-packed [128, W*64]; P2 gathers it the same way with
    lane-mask x Dinv scales, reduces per node, and scatter-adds fp32 rows
    into ARIN quarters (dst idx < 32768). Pad rows add 0 to an absent row.
  - ReduceScatter(ARIN) -> per-core node shard; stats (C|S) on the shard +
    tiny AllReduce; BN folded into W' and c'; apply (transpose+matmul+
    softplus) on the shard only; AllGather fp16 shards -> next HTAB_P.
  - Pooling fused into last apply via on-device graph-id compare matmuls;
    AllReduce of pooled [64, Gpad]; FC head replicated.
"""
import os
import sys
sys.path.insert(0, "/opt/trn_rl_repo")
import numpy as np

LAST_EXEC_NS = None

P = 128
NCORES = 8
F = 64
SLOTCAP = 4096
GICOLS = 8      # cols per gather/scatter instruction (1024 descs, ring-limited)
SCOLS = SLOTCAP // P   # 32 gather cols per chunk


def _wrap16(v):
    n = len(v)
    assert n % 16 == 0
    w = np.asarray(v, np.int16).reshape(n // 16, 16).T
    return np.tile(w, (8, 1))


def _cumcount(keys):
    """cumcount of equal consecutive-group keys (any int array)."""
    I = len(keys)
    o = np.argsort(keys, kind="stable")
    ks = keys[o]
    newg = np.concatenate([[True], ks[1:] != ks[:-1]])
    st = np.nonzero(newg)[0]
    d = np.empty(I, np.int64)
    d[o] = np.arange(I) - np.repeat(st, np.diff(np.append(st, I)))
    return d


def _chunk_windows(Dw, cap_cols):
    """Greedy consecutive windows with sum(D) <= cap_cols.
    Returns list of (w0, w1)."""
    out = []
    w0 = 0
    W = len(Dw)
    while w0 < W:
        w1, tot = w0, 0
        while w1 < W and tot + Dw[w1] <= cap_cols:
            tot += Dw[w1]
            w1 += 1
        assert w1 > w0, f"window D={Dw[w0]} exceeds cap {cap_cols}"
        out.append((w0, w1))
        w0 = w1
    return out


def _runs(Dw, w0, w1):
    """Merge consecutive equal-D windows into (rel_col0, nwin, D) runs."""
    runs = []
    c0 = 0
    for w in range(w0, w1):
        D = int(Dw[w])
        if runs and runs[-1][2] == D:
            runs[-1] = (runs[-1][0], runs[-1][1] + 1, D)
        else:
            runs.append((c0, 1, D))
        c0 += D
    return runs


def host_prep2(node_idx, edge_idx, batch, N, G):
    I = len(node_idx)
    E = int(edge_idx.max()) + 1
    node_idx = np.asarray(node_idx, np.int64)
    edge_idx = np.asarray(edge_idx, np.int64)
    batch = np.asarray(batch, np.int64)

    Npad = ((N + 1023) // 1024) * 1024
    NS = Npad // NCORES
    NP4 = Npad // 4
    NQ4 = Npad // 4            # scatter quarter size (rows per region)
    assert NP4 <= 32767 and NQ4 <= 32767

    deg_e = np.bincount(edge_idx, minlength=E)
    deg_n = np.bincount(node_idx, minlength=N)
    Binv = np.where(deg_e > 0, 1.0 / np.maximum(deg_e, 1), 0.0).astype(np.float32)
    Dinv = np.where(deg_n > 0, 1.0 / np.maximum(deg_n, 1), 0.0).astype(np.float32)

    # ---- edge deal: global degree-desc order, round robin
    live = np.nonzero(deg_e > 0)[0]
    order = live[np.argsort(-deg_e[live], kind="stable")]
    NL_ = len(order)
    core_of_edge = np.full(E, -1, np.int64)
    rank_of_edge = np.full(E, -1, np.int64)
    core_of_edge[order] = np.arange(NL_) % NCORES
    rank_of_edge[order] = np.arange(NL_) // NCORES
    ECmax = (NL_ + NCORES - 1) // NCORES
    W1 = ((ECmax + 127) // 128 + 3) // 4 * 4    # windows, mult of 4
    assert 32 * W1 <= 32767

    # D per window: degree of the first (max) rank in the window, max over cores
    Dw1 = np.ones(W1, np.int64)
    for w in range(W1):
        gi = 128 * w * NCORES
        if gi < NL_:
            Dw1[w] = max(1, int(deg_e[order[gi]]))
    assert Dw1.max() <= SCOLS
    colbase1 = np.concatenate([[0], np.cumsum(Dw1)]).astype(np.int64)
    TOTC1 = int(colbase1[-1])

    chunks1 = []
    for (w0, w1) in _chunk_windows(Dw1, SCOLS):
        chunks1.append(dict(w0=w0, w1=w1, c0=int(colbase1[w0]),
                            cols=int(colbase1[w1] - colbase1[w0]),
                            runs=_runs(Dw1, w0, w1)))

    pc = core_of_edge[edge_idx]
    pr = rank_of_edge[edge_idx]
    pp, pw = pr % 128, pr // 128
    epos = pp * W1 + pw

    d1 = _cumcount(edge_idx)
    col1 = colbase1[pw] + d1
    idx1 = np.zeros((NCORES, TOTC1 * 128), np.int16)
    idx1[pc, col1 * 128 + pp] = (node_idx // 4).astype(np.int16)
    sc1 = np.zeros((NCORES, P, 4 * TOTC1), np.float16)
    sc1[pc, pp, 4 * col1 + node_idx % 4] = Binv[edge_idx].astype(np.float16)

    # ---- P2: per-quarter, per-core count-sorted nodes
    cnt = np.zeros((NCORES, N), np.int32)
    np.add.at(cnt, (pc, node_idx), 1)
    rank2 = np.full((NCORES, N), -1, np.int64)
    Dw2_all, wbase, dump = [], [0], np.zeros((NCORES, 4), np.int64)
    q_of_node = node_idx // NQ4
    for q in range(4):
        nlo, nhi = q * NQ4, min((q + 1) * NQ4, N)
        per_core = []
        for c in range(NCORES):
            cs = cnt[c, nlo:nhi]
            pres = np.nonzero(cs > 0)[0]
            o = pres[np.argsort(-cs[pres], kind="stable")]
            per_core.append(o)
            rank2[c, nlo + o] = np.arange(len(o))
            absent = np.nonzero(cs == 0)[0]
            assert len(absent) > 0
            dump[c, q] = absent[0]
        L = max(len(o) for o in per_core)
        Wq = max(1, (L + 127) // 128)
        Dq = np.ones(Wq, np.int64)
        for c in range(NCORES):
            o = per_core[c]
            for w in range((len(o) + 127) // 128):
                Dq[w] = max(Dq[w], int(cnt[c, nlo + o[128 * w]]))
        assert Dq.max() <= SCOLS
        Dw2_all.append(Dq)
        wbase.append(wbase[-1] + Wq)
    Dw2 = np.concatenate(Dw2_all)
    W2T = int(wbase[-1])
    colbase2 = np.concatenate([[0], np.cumsum(Dw2)]).astype(np.int64)
    TOTC2 = int(colbase2[-1])
    ROWS2 = W2T * 128

    chunks2 = []
    for q in range(4):
        for (w0, w1) in _chunk_windows(Dw2_all[q], SCOLS):
            gw0, gw1 = wbase[q] + w0, wbase[q] + w1
            chunks2.append(dict(q=q, w0=gw0, w1=gw1, c0=int(colbase2[gw0]),
                                cols=int(colbase2[gw1] - colbase2[gw0]),
                                runs=_runs(Dw2, gw0, gw1)))

    rr = rank2[pc, node_idx]
    assert (rr >= 0).all()
    p2, w2q = rr % 128, rr // 128
    gw = np.asarray(wbase, np.int64)[q_of_node] + w2q
    d2 = _cumcount(pc * np.int64(N) + node_idx)
    col2 = colbase2[gw] + d2
    idx2 = np.zeros((NCORES, TOTC2 * 128), np.int16)
    idx2[pc, col2 * 128 + p2] = (epos // 4).astype(np.int16)
    sc2 = np.zeros((NCORES, P, 4 * TOTC2), np.float16)
    sc2[pc, p2, 4 * col2 + epos % 4] = Dinv[node_idx].astype(np.float16)

    sx2 = np.zeros((NCORES, ROWS2), np.int16)
    for c in range(NCORES):
        for q in range(4):
            lo, hi = wbase[q] * 128, wbase[q + 1] * 128
            sx2[c, lo:hi] = dump[c, q]
    # real rows
    sel = np.ones(I, bool)
    first = d2 == 0
    sx2[pc[first], gw[first] * 128 + p2[first]] = (
        node_idx[first] - q_of_node[first] * NQ4).astype(np.int16)

    # ---- pooling: batch value per shard tile slot; graph iota
    Gpad = ((G + 127) // 128) * 128
    NTS = NS // 128
    bt = np.full((NCORES, P, NTS), 2 * Gpad, np.float32)
    for c in range(NCORES):
        lo = c * NS
        real = min(NS, max(0, N - lo))
        vals = np.full(NS, 2 * Gpad, np.float32)
        vals[:real] = batch[lo:lo + real]
        bt[c] = vals.reshape(NTS, P).T
    cnt_g = np.bincount(batch, minlength=Gpad).astype(np.float32)
    cinv = np.zeros((1, Gpad), np.float32)
    cinv[0, :Gpad] = 1.0 / np.maximum(cnt_g, 1.0)
    giota = np.arange(Gpad, dtype=np.float32).reshape(1, Gpad)

    return dict(
        N=N, E=E, I=I, G=G, Npad=Npad, NS=NS, NP4=NP4, NQ4=NQ4, Gpad=Gpad,
        W1=W1, TOTC1=TOTC1, chunks1=chunks1, Dw1=Dw1,
        W2T=W2T, TOTC2=TOTC2, ROWS2=ROWS2, chunks2=chunks2, Dw2=Dw2,
        idx1=idx1, sc1=sc1, idx2=idx2, sc2=sc2, sx2=sx2,
        bt=bt, cinv=cinv, giota=giota,
        Binv=Binv, Dinv=Dinv, core_of_edge=core_of_edge,
        rank_of_edge=rank_of_edge, wbase=wbase,
    )


# ----------------------------------------------------------------------------
def build_nc2(pre, AD, NL=3, HD=128):
    import concourse.bass as bass
    import concourse.mybir as mybir
    from concourse import bacc
    from concourse.tile import TileContext
    from concourse.masks import make_identity

    AF = mybir.ActivationFunctionType
    OP = mybir.AluOpType
    f32 = mybir.dt.float32
    f16 = mybir.dt.float16
    i16 = mybir.dt.int16

    N = pre["N"]; G = pre["G"]; Gpad = pre["Gpad"]
    Npad = pre["Npad"]; NS = pre["NS"]; NP4 = pre["NP4"]; NQ4 = pre["NQ4"]
    W1 = pre["W1"]; TOTC1 = pre["TOTC1"]
    W2T = pre["W2T"]; TOTC2 = pre["TOTC2"]; ROWS2 = pre["ROWS2"]
    NTS = NS // 128
    GW = Gpad // 128

    nc = bacc.Bacc(num_devices=NCORES)
    xTs = nc.dram_tensor("xTs", [AD, NS], f32, kind="ExternalInput")
    W_emb = nc.dram_tensor("W_emb", [AD, F], f32, kind="ExternalInput")
    b_emb = nc.dram_tensor("b_emb", [1, F], f32, kind="ExternalInput")
    convW2 = nc.dram_tensor("convW2", [NL * P, F], f32, kind="ExternalInput")
    bn_gc = nc.dram_tensor("bn_gc", [F, NL], f32, kind="ExternalInput")
    bn_bc = nc.dram_tensor("bn_bc", [F, NL], f32, kind="ExternalInput")
    fc_W = nc.dram_tensor("fc_W", [F, HD], f32, kind="ExternalInput")
    fc_b = nc.dram_tensor("fc_b", [1, HD], f32, kind="ExternalInput")
    fco_W = nc.dram_tensor("fco_W", [HD, 1], f32, kind="ExternalInput")
    fco_b = nc.dram_tensor("fco_b", [1, 1], f32, kind="ExternalInput")
    p1_idx = nc.dram_tensor("p1_idx", [P, TOTC1 * 8], i16, kind="ExternalInput")
    p1_sc = nc.dram_tensor("p1_sc", [P, 4 * TOTC1], f16, kind="ExternalInput")
    p2_idx = nc.dram_tensor("p2_idx", [P, TOTC2 * 8], i16, kind="ExternalInput")
    p2_sc = nc.dram_tensor("p2_sc", [P, 4 * TOTC2], f16, kind="ExternalInput")
    p2_sx = nc.dram_tensor("p2_sx", [P, ROWS2 // 16], i16, kind="ExternalInput")
    btT = nc.dram_tensor("btT", [P, NTS], f32, kind="ExternalInput")
    cinv = nc.dram_tensor("cinv", [1, Gpad], f32, kind="ExternalInput")
    giota = nc.dram_tensor("giota", [1, Gpad], f32, kind="ExternalInput")

    HTAB_P = nc.dram_tensor("HTAB_P", [NP4, 256], f16, addr_space="Shared")
    HSL = [nc.dram_tensor(f"HS{b}", [NS, F], f16) for b in range(NL + 1)]
    EF_P = nc.dram_tensor("EF_P", [P, W1 * F], f16)
    ARIN = [nc.dram_tensor(f"ARIN{b}", [Npad, F], f32) for b in range(NL)]
    ARO_SL = [nc.dram_tensor(f"ARO_S{b}", [NS, F], f32) for b in range(NL)]
    CS_IL = [nc.dram_tensor(f"CS_I{b}", [F, F + 1], f32) for b in range(NL)]
    CS_OL = [nc.dram_tensor(f"CS_O{b}", [F, F + 1], f32, addr_space="Shared")
             for b in range(NL)]
    PACC_I = nc.dram_tensor("PACC_I", [F, Gpad], f32)
    PACC_O = nc.dram_tensor("PACC_O", [F, Gpad], f32, addr_space="Shared")
    OUT = nc.dram_tensor("OUT", [Gpad, 1], f32, kind="ExternalOutput")
    DBG = bool(int(os.environ.get("KDBG", "0")))
    if DBG:
        D_EF = nc.dram_tensor("D_EF", [P, W1 * F], f16, kind="ExternalOutput")
        D_ARO = [nc.dram_tensor(f"D_ARO{b}", [NS, F], f32, kind="ExternalOutput")
                 for b in range(NL)]
        D_CS = [nc.dram_tensor(f"D_CS{b}", [F, F + 1], f32, kind="ExternalOutput")
                for b in range(NL)]
        D_HS = [nc.dram_tensor(f"D_HS{b}", [NS, F], f16, kind="ExternalOutput")
                for b in range(NL + 1)]
        D_CST = [nc.dram_tensor(f"D_CST{b}", [F, F + 1], f32, kind="ExternalOutput")
                 for b in range(NL)]
        D_VAR = [nc.dram_tensor(f"D_VAR{b}", [F, 4], f32, kind="ExternalOutput")
                 for b in range(NL)]

    groups = [list(range(NCORES))]
    efv = EF_P[:, :].rearrange("p (r f) -> (p r) f", f=256)
    KB2 = int(os.environ.get("KB2", "9"))

    with TileContext(nc) as tc:
        with (
            tc.tile_pool(name="const", bufs=1) as cpool,
            tc.tile_pool(name="gbuf", bufs=2) as gpool,
            tc.tile_pool(name="stage", bufs=2) as spool,
            tc.tile_pool(name="small", bufs=2) as mpool,
            tc.tile_pool(name="dense", bufs=3) as dpool,
            tc.tile_pool(name="psum", bufs=2, space="PSUM") as ppool,
            tc.tile_pool(name="pstr", bufs=2, space="PSUM") as tpool,
            tc.tile_pool(name="psA", bufs=1, space="PSUM") as psA,
            tc.tile_pool(name="psB", bufs=1, space="PSUM") as psB,
        ):
            ident = cpool.tile([P, P], f32)
            make_identity(nc, ident[:])
            ones_col = cpool.tile([P, 1], f32)
            nc.gpsimd.memset(ones_col[:], 1.0)
            ones_row = cpool.tile([1, P], f32)
            nc.gpsimd.memset(ones_row[:], 1.0)
            zt = cpool.tile([P, 4096], f32)
            nc.gpsimd.memset(zt[:], 0.0)

            # giota broadcast [P, Gpad] via PE
            git = mpool.tile([1, Gpad], f32)
            nc.sync.dma_start(out=git[:], in_=giota[:, :])
            gps = ppool.tile([P, Gpad], f32, space="PSUM", tag="ps")
            nc.tensor.matmul(out=gps[:], lhsT=ones_row[:], rhs=git[:],
                             start=True, stop=True)
            giota_b = cpool.tile([P, Gpad], f32)
            nc.vector.tensor_copy(out=giota_b[:], in_=gps[:])
            btc = cpool.tile([P, NTS], f32)
            nc.sync.dma_start(out=btc[:], in_=btT[:, :])

            # ---- zero ARIN0/ARIN1
            def zero_arin(t):
                flat = t[:, :].rearrange("(p k) f -> p (k f)", p=P)
                KT = (Npad // P) * F
                for o in range(0, KT, 4096):
                    kk = min(4096, KT - o)
                    nc.sync.dma_start(out=flat[:, o:o + kk], in_=zt[:, :kk])
            for t in ARIN:
                zero_arin(t)

            # ---- embedding -> HS -> AG
            wemb_t = cpool.tile([AD, F], f32)
            nc.sync.dma_start(out=wemb_t[:], in_=W_emb[:, :])
            bemb_t = mpool.tile([1, F], f32)
            nc.sync.dma_start(out=bemb_t[:], in_=b_emb[:, :])
            bps = ppool.tile([P, F], f32, space="PSUM", tag="ps")
            nc.tensor.matmul(out=bps[:], lhsT=ones_row[:], rhs=bemb_t[:],
                             start=True, stop=True)
            bemb4 = cpool.tile([P, 4, F], f32)
            for j in range(4):
                nc.vector.tensor_copy(out=bemb4[:, j, :], in_=bps[:])
            hs3 = HSL[0][:, :].rearrange("(t p) f -> p t f", p=P)
            for t0 in range(0, NTS, 4):
                tt = min(4, NTS - t0)
                xc = gpool.tile([AD, 4 * P], f32, tag="xc")
                nc.sync.dma_start(out=xc[:, :tt * P],
                                  in_=xTs[:, t0 * P:(t0 + tt) * P])
                eps_ = ppool.tile([P, 4 * F], f32, space="PSUM", tag="ps")
                for j in range(tt):
                    nc.tensor.matmul(out=eps_[:, j * F:(j + 1) * F],
                                     lhsT=xc[:, j * P:(j + 1) * P],
                                     rhs=wemb_t[:], start=True, stop=True)
                hb = spool.tile([P, 4, F], f16, tag="hb")
                nc.vector.tensor_tensor(
                    out=hb[:, :tt, :],
                    in0=eps_[:, :tt * F].rearrange("p (t f) -> p t f", f=F),
                    in1=bemb4[:, :tt, :], op=OP.add)
                nc.sync.dma_start(out=hs3[:, t0:t0 + tt, :], in_=hb[:, :tt, :])
            nc.gpsimd.collective_compute(
                "AllGather", OP.bypass, replica_groups=groups,
                ins=[HSL[0][:, :]], outs=[HTAB_P[:, :]])

            # ---- layers
            for l in range(NL):
                cur = ARIN[l]
                ARO_S = ARO_SL[l]
                CS_I, CS_O = CS_IL[l], CS_OL[l]
                hs3 = HSL[l + 1][:, :].rearrange("(t p) f -> p t f", p=P)
                # == P1: gather HTAB_P -> EF_P
                for ch in pre["chunks1"]:
                    cols = ch["cols"]; nwt = ch["w1"] - ch["w0"]
                    it = gpool.tile([P, SLOTCAP // 16], i16, tag="it")
                    nc.sync.dma_start(
                        out=it[:, :cols * 8],
                        in_=p1_idx[:, ch["c0"] * 8:(ch["c0"] + cols) * 8])
                    g = gpool.tile([P, SCOLS, 256], f16, tag="g")
                    for b0 in range(0, cols, GICOLS):
                        bb = min(GICOLS, cols - b0)
                        nc.gpsimd.dma_gather(
                            out_ap=g[:, b0:b0 + bb, :], in_ap=HTAB_P[:, :],
                            idxs_ap=it[:, b0 * 8:(b0 + bb) * 8],
                            num_idxs=bb * 128,
                            num_idxs_reg=bb * 128, elem_size=256)
                    st = gpool.tile([P, 4 * SCOLS], f16, tag="st")
                    nc.sync.dma_start(
                        out=st[:, :4 * cols],
                        in_=p1_sc[:, 4 * ch["c0"]:4 * (ch["c0"] + cols)])
                    gs = gpool.tile([P, 4 * SCOLS, F], f16, tag="gs")
                    nc.vector.tensor_tensor(
                        out=gs[:, :4 * cols, :],
                        in0=g[:, :cols, :].rearrange("p c (l f) -> p (c l) f", l=4),
                        in1=st[:, :4 * cols].to_broadcast([P, 4 * cols, F]),
                        op=OP.mult)
                    red = spool.tile([P, SCOLS, F], f16, tag="red")
                    wi = 0
                    with nc.allow_low_precision(reason="f16 edge mean"):
                        for (c0, rnw, D) in ch["runs"]:
                            src = gs[:, 4 * c0:4 * (c0 + rnw * D), :].rearrange(
                                "p (w d) f -> p w f d", w=rnw, d=4 * D)
                            nc.vector.tensor_reduce(
                                out=red[:, wi:wi + rnw, :], in_=src,
                                axis=mybir.AxisListType.X, op=OP.add)
                            wi += rnw
                    nc.sync.dma_start(
                        out=EF_P[:, ch["w0"] * F:ch["w1"] * F].rearrange(
                            "p (w f) -> p w f", f=F),
                        in_=red[:, :nwt, :])

                # == P2: gather EF_P -> scatter ARIN quarters
                for ch in (pre["chunks2"] if KB2 >= 2 else []):
                    cols = ch["cols"]; nwt = ch["w1"] - ch["w0"]
                    it = gpool.tile([P, SLOTCAP // 16], i16, tag="it")
                    nc.sync.dma_start(
                        out=it[:, :cols * 8],
                        in_=p2_idx[:, ch["c0"] * 8:(ch["c0"] + cols) * 8])
                    g = gpool.tile([P, SCOLS, 256], f16, tag="g")
                    for b0 in range(0, cols, GICOLS):
                        bb = min(GICOLS, cols - b0)
                        nc.gpsimd.dma_gather(
                            out_ap=g[:, b0:b0 + bb, :], in_ap=efv,
                            idxs_ap=it[:, b0 * 8:(b0 + bb) * 8],
                            num_idxs=bb * 128,
                            num_idxs_reg=bb * 128, elem_size=256)
                    st = gpool.tile([P, 4 * SCOLS], f16, tag="st")
                    nc.sync.dma_start(
                        out=st[:, :4 * cols],
                        in_=p2_sc[:, 4 * ch["c0"]:4 * (ch["c0"] + cols)])
                    gs = gpool.tile([P, 4 * SCOLS, F], f16, tag="gs")
                    nc.vector.tensor_tensor(
                        out=gs[:, :4 * cols, :],
                        in0=g[:, :cols, :].rearrange("p c (l f) -> p (c l) f", l=4),
                        in1=st[:, :4 * cols].to_broadcast([P, 4 * cols, F]),
                        op=OP.mult)
                    red = spool.tile([P, SCOLS, F], f32, tag="red32")
                    wi = 0
                    for (c0, rnw, D) in ch["runs"]:
                        src = gs[:, 4 * c0:4 * (c0 + rnw * D), :].rearrange(
                            "p (w d) f -> p w f d", w=rnw, d=4 * D)
                        nc.vector.tensor_reduce(
                            out=red[:, wi:wi + rnw, :], in_=src,
                            axis=mybir.AxisListType.X, op=OP.add)
                        wi += rnw
                    sx = gpool.tile([P, SLOTCAP // 16], i16, tag="sx")
                    nc.sync.dma_start(
                        out=sx[:, :nwt * 8],
                        in_=p2_sx[:, ch["w0"] * 8:ch["w1"] * 8])
                    q = ch["q"]
                    for o in range(0, nwt, GICOLS):
                        oo = min(GICOLS, nwt - o)
                        nc.gpsimd.dma_scatter_add(
                            out_ap=cur[q * NQ4:(q + 1) * NQ4, :],
                            in_ap=red[:, o:o + oo, :],
                            idxs_ap=sx[:, o * 8:(o + oo) * 8],
                            num_idxs=oo * 128, num_idxs_reg=oo * 128,
                            elem_size=F)

                # == ReduceScatter -> ARO_S
                if KB2 < 3:
                    continue
                nc.gpsimd.collective_compute(
                    "ReduceScatter", OP.add, replica_groups=groups,
                    ins=[cur[:, :]], outs=[ARO_S[:, :]])

                # == stats on shard: C | S
                if KB2 < 4:
                    continue
                o3 = ARO_S[:, :].rearrange("(t p) f -> p t f", p=P)
                Cps = psA.tile([F, F], f32, space="PSUM", tag="C")
                Sps = psA.tile([F, 1], f32, space="PSUM", tag="S")
                for t in range(NTS):
                    rt = dpool.tile([P, F], f32, tag="rt")
                    nc.sync.dma_start(out=rt[:], in_=o3[:, t, :])
                    nc.tensor.matmul(out=Cps[:], lhsT=rt[:], rhs=rt[:],
                                     start=(t == 0), stop=(t == NTS - 1))
                    nc.tensor.matmul(out=Sps[:], lhsT=rt[:], rhs=ones_col[:],
                                     start=(t == 0), stop=(t == NTS - 1))
                cs_t = mpool.tile([F, F + 1], f32)
                nc.vector.tensor_copy(out=cs_t[:, :F], in_=Cps[:])
                nc.vector.tensor_copy(out=cs_t[:, F:F + 1], in_=Sps[:])
                nc.sync.dma_start(out=CS_I[:, :], in_=cs_t[:])
                nc.gpsimd.collective_compute(
                    "AllReduce", OP.add, replica_groups=groups,
                    ins=[CS_I[:, :]], outs=[CS_O[:, :]])
                cst = mpool.tile([F, F + 1], f32)
                nc.sync.dma_start(out=cst[:], in_=CS_O[:, :])
                if DBG:
                    nc.sync.dma_start(out=D_CST[l][:, :], in_=cst[:])

                # == BN fold: a, cfin; Wp2 = W * a_row; cb4 = cfin rows
                Wl = cpool.tile([P, F], f32, tag="Wl")
                nc.sync.dma_start(out=Wl[:], in_=convW2[l * P:(l + 1) * P, :])
                gcol = mpool.tile([F, 1], f32)
                nc.sync.dma_start(out=gcol[:], in_=bn_gc[:, l:l + 1])
                btcol = mpool.tile([F, 1], f32)
                nc.sync.dma_start(out=btcol[:], in_=bn_bc[:, l:l + 1])
                mps = ppool.tile([F, 1], f32, space="PSUM", tag="ps")
                nc.tensor.matmul(out=mps[:], lhsT=Wl[:F, :], rhs=cst[:, F:F + 1],
                                 start=True, stop=True)
                mrw = mpool.tile([F, 1], f32)
                nc.scalar.activation(out=mrw[:], in_=mps[:], func=AF.Copy,
                                     scale=1.0 / N)
                # centered covariance BEFORE contracting with W: avoids the
                # e2 - mean^2 catastrophic cancellation (var << mean^2 here).
                mxc = mpool.tile([F, 1], f32)
                nc.scalar.activation(out=mxc[:], in_=cst[:, F:F + 1], func=AF.Copy,
                                     scale=1.0 / N)
                mxps = ppool.tile([1, F], f32, space="PSUM", tag="ps")
                nc.tensor.matmul(out=mxps[:], lhsT=mxc[:], rhs=ident[:F, :F],
                                 start=True, stop=True)
                mxr = mpool.tile([1, F], f32)
                nc.vector.tensor_copy(out=mxr[:], in_=mxps[:])
                mmps = ppool.tile([F, F], f32, space="PSUM", tag="ps")
                nc.tensor.matmul(out=mmps[:], lhsT=mxr[:], rhs=mxr[:],
                                 start=True, stop=True)
                Cn = mpool.tile([F, F], f32)
                nc.scalar.activation(out=Cn[:], in_=cst[:, :F], func=AF.Copy,
                                     scale=1.0 / N)
                nc.vector.tensor_tensor(out=Cn[:], in0=Cn[:], in1=mmps[:],
                                        op=OP.subtract)
                t1ps = ppool.tile([F, F], f32, space="PSUM", tag="ps")
                nc.tensor.matmul(out=t1ps[:], lhsT=Cn[:], rhs=Wl[:F, :],
                                 start=True, stop=True)
                wt1 = mpool.tile([F, F], f32)
                nc.vector.tensor_tensor(out=wt1[:], in0=t1ps[:], in1=Wl[:F, :],
                                        op=OP.mult)
                e2ps = ppool.tile([F, 1], f32, space="PSUM", tag="ps")
                nc.tensor.matmul(out=e2ps[:], lhsT=wt1[:], rhs=ones_col[:F, :],
                                 start=True, stop=True)
                var = mpool.tile([F, 1], f32)
                nc.vector.tensor_scalar(out=var[:], in0=e2ps[:], scalar1=0.0,
                                        scalar2=1e-5, op0=OP.max, op1=OP.add)
                lnv = mpool.tile([F, 1], f32)
                nc.scalar.activation(out=lnv[:], in_=var[:], func=AF.Ln)
                rstd = mpool.tile([F, 1], f32)
                nc.scalar.activation(out=rstd[:], in_=lnv[:], func=AF.Exp, scale=-0.5)
                a_ = mpool.tile([F, 1], f32)
                nc.vector.tensor_tensor(out=a_[:], in0=gcol[:], in1=rstd[:], op=OP.mult)
                if DBG:
                    dvt = mpool.tile([F, 4], f32)
                    nc.vector.tensor_copy(out=dvt[:, 0:1], in_=var[:])
                    nc.vector.tensor_copy(out=dvt[:, 1:2], in_=a_[:])
                    nc.vector.tensor_copy(out=dvt[:, 2:3], in_=mrw[:])
                    nc.vector.tensor_copy(out=dvt[:, 3:4], in_=mxc[:])
                    nc.sync.dma_start(out=D_VAR[l][:, :], in_=dvt[:])
                am = mpool.tile([F, 1], f32)
                nc.vector.tensor_tensor(out=am[:], in0=a_[:], in1=mrw[:], op=OP.mult)
                cfin = mpool.tile([F, 1], f32)
                nc.vector.tensor_tensor(out=cfin[:], in0=btcol[:], in1=am[:],
                                        op=OP.subtract)
                arps = ppool.tile([1, F], f32, space="PSUM", tag="ps")
                nc.tensor.matmul(out=arps[:], lhsT=a_[:], rhs=ident[:F, :F],
                                 start=True, stop=True)
                arow = mpool.tile([1, F], f32)
                nc.vector.tensor_copy(out=arow[:], in_=arps[:])
                abps = ppool.tile([P, F], f32, space="PSUM", tag="ps")
                nc.tensor.matmul(out=abps[:], lhsT=ones_row[:], rhs=arow[:],
                                 start=True, stop=True)
                Wp2 = cpool.tile([P, F], f32, tag="Wp")
                nc.vector.tensor_tensor(out=Wp2[:], in0=Wl[:], in1=abps[:], op=OP.mult)
                crps = ppool.tile([1, F], f32, space="PSUM", tag="ps")
                nc.tensor.matmul(out=crps[:], lhsT=cfin[:], rhs=ident[:F, :F],
                                 start=True, stop=True)
                crow = mpool.tile([1, F], f32)
                nc.vector.tensor_copy(out=crow[:], in_=crps[:])
                cbps = ppool.tile([P, F], f32, space="PSUM", tag="ps")
                nc.tensor.matmul(out=cbps[:], lhsT=ones_row[:], rhs=crow[:],
                                 start=True, stop=True)
                cb4 = cpool.tile([P, 4, F], f32, tag="cb")
                for j in range(4):
                    nc.vector.tensor_copy(out=cb4[:, j, :], in_=cbps[:])

                # == apply on shard (+ fused pooling on last layer)
                if KB2 < 5:
                    continue
                last = (l == NL - 1)
                if last:
                    pacc = psB.tile([F, Gpad], f32, space="PSUM", tag="pacc")
                for t0 in range(0, NTS, 4):
                    kk = min(4, NTS - t0)
                    rc = dpool.tile([P, 4, F], f32, tag="rc")
                    nc.sync.dma_start(out=rc[:, :kk, :], in_=o3[:, t0:t0 + kk, :])
                    yps = ppool.tile([P, 4 * F], f32, space="PSUM", tag="ps")
                    for j in range(kk):
                        trp = tpool.tile([F, P], f32, space="PSUM", tag="tr")
                        nc.tensor.transpose(out=trp[:], in_=rc[:, j, :],
                                            identity=ident[:])
                        trs = dpool.tile([F, P], f32, tag="trs")
                        nc.vector.tensor_copy(out=trs[:], in_=trp[:])
                        nc.tensor.matmul(out=yps[:, j * F:(j + 1) * F],
                                         lhsT=trs[:], rhs=Wp2[0:F, :],
                                         start=True, stop=True)
                    yb = dpool.tile([P, 4 * F], f32, tag="yb")
                    nc.vector.tensor_tensor(
                        out=yb[:, :kk * F], in0=yps[:, :kk * F],
                        in1=cb4[:, :kk, :].rearrange("p t f -> p (t f)"),
                        op=OP.add)
                    ex = dpool.tile([P, 4 * F], f32, tag="ex")
                    nc.scalar.activation(out=ex[:, :kk * F], in_=yb[:, :kk * F],
                                         func=AF.Exp)
                    hc = dpool.tile([P, 4, F], f16, tag="hc")
                    nc.scalar.activation(
                        out=hc[:, :kk, :].rearrange("p t f -> p (t f)"),
                        in_=ex[:, :kk * F], func=AF.Ln, bias=1.0, scale=1.0)
                    nc.sync.dma_start(out=hs3[:, t0:t0 + kk, :], in_=hc[:, :kk, :])
                    if last:
                        for j in range(kk):
                            t = t0 + j
                            bq = dpool.tile([P, Gpad], f16, tag="bq")
                            nc.vector.tensor_tensor(
                                out=bq[:], in0=btc[:, t:t + 1].to_broadcast([P, Gpad]),
                                in1=giota_b[:], op=OP.is_equal)
                            nc.tensor.matmul(out=pacc[:], lhsT=hc[:, j, :],
                                             rhs=bq[:], start=(t == 0),
                                             stop=(t == NTS - 1))
                nc.gpsimd.collective_compute(
                    "AllGather", OP.bypass, replica_groups=groups,
                    ins=[HSL[l + 1][:, :]], outs=[HTAB_P[:, :]])

            # ---- pooled AllReduce + scale + FC head
            if KB2 < 5:
                dz = dpool.tile([P, 1], f32, tag="dz")
                nc.gpsimd.memset(dz[:], 0.0)
                for w in range(GW):
                    nc.sync.dma_start(out=OUT[w * P:(w + 1) * P, :], in_=dz[:])
            pc_t = mpool.tile([F, Gpad], f32)
            if KB2 < 5:
                nc.gpsimd.memset(pc_t[:], 0.0)
            else:
                nc.vector.tensor_copy(out=pc_t[:], in_=pacc[:])
            nc.sync.dma_start(out=PACC_I[:, :], in_=pc_t[:])
            nc.gpsimd.collective_compute(
                "AllReduce", OP.add, replica_groups=groups,
                ins=[PACC_I[:, :]], outs=[PACC_O[:, :]])
            psum_t = mpool.tile([F, Gpad], f32)
            nc.sync.dma_start(out=psum_t[:], in_=PACC_O[:, :])
            civ = mpool.tile([1, Gpad], f32)
            nc.sync.dma_start(out=civ[:], in_=cinv[:, :])
            cps = ppool.tile([F, Gpad], f32, space="PSUM", tag="ps")
            nc.tensor.matmul(out=cps[:], lhsT=ones_row[:, :F],
                             rhs=civ[:], start=True, stop=True)
            civ_b = mpool.tile([F, Gpad], f32)
            nc.vector.tensor_copy(out=civ_b[:], in_=cps[:])
            poolacc = mpool.tile([F, Gpad], f32)
            nc.vector.tensor_tensor(out=poolacc[:], in0=psum_t[:], in1=civ_b[:],
                                    op=OP.mult)

            fcw_t = cpool.tile([F, HD], f32)
            nc.sync.dma_start(out=fcw_t[:], in_=fc_W[:, :])
            fcb_t = mpool.tile([1, HD], f32)
            nc.sync.dma_start(out=fcb_t[:], in_=fc_b[:, :])
            fcbps = ppool.tile([P, HD], f32, space="PSUM", tag="ps")
            nc.tensor.matmul(out=fcbps[:], lhsT=ones_row[:], rhs=fcb_t[:],
                             start=True, stop=True)
            fcb_b = cpool.tile([P, HD], f32)
            nc.vector.tensor_copy(out=fcb_b[:], in_=fcbps[:])
            if DBG:
                def dump_rows(dst, srct, rows, dt):
                    s3 = srct[:, :].rearrange("(t p) f -> p t f", p=P)
                    d3 = dst[:, :].rearrange("(t p) f -> p t f", p=P)
                    for t0 in range(0, rows // P, 8):
                        kk = min(8, rows // P - t0)
                        tdb = spool.tile([P, 8, F], dt, tag="tdb" + str(dt))
                        nc.sync.dma_start(out=tdb[:, :kk, :], in_=s3[:, t0:t0 + kk, :])
                        nc.sync.dma_start(out=d3[:, t0:t0 + kk, :], in_=tdb[:, :kk, :])
                for b in range(NL):
                    dump_rows(D_ARO[b], ARO_SL[b], NS, f32)
                    tcs = mpool.tile([F, F + 1], f32)
                    nc.sync.dma_start(out=tcs[:], in_=CS_OL[b][:, :])
                    nc.sync.dma_start(out=D_CS[b][:, :], in_=tcs[:])
                for b in range(NL + 1):
                    dump_rows(D_HS[b], HSL[b], NS, f16)
                for o in range(0, W1 * F, 2048):
                    kk = min(2048, W1 * F - o)
                    tdb = spool.tile([P, 2048], f16, tag="tdbe")
                    nc.sync.dma_start(out=tdb[:, :kk], in_=EF_P[:, o:o + kk])
                    nc.sync.dma_start(out=D_EF[:, o:o + kk], in_=tdb[:, :kk])
            fcow_t = cpool.tile([HD, 1], f32)
            nc.sync.dma_start(out=fcow_t[:], in_=fco_W[:, :])
            fcob_t = mpool.tile([1, 1], f32)
            nc.sync.dma_start(out=fcob_t[:], in_=fco_b[:, :])
            fcobps = ppool.tile([P, 1], f32, space="PSUM", tag="ps")
            nc.tensor.matmul(out=fcobps[:], lhsT=ones_row[:], rhs=fcob_t[:],
                             start=True, stop=True)
            fcob_b = cpool.tile([P, 1], f32)
            nc.vector.tensor_copy(out=fcob_b[:], in_=fcobps[:])
            for w in range(GW):
                gts = dpool.tile([F, P], f32, tag="gts")
                ge_ = dpool.tile([F, P], f32, tag="ge")
                nc.scalar.activation(out=ge_[:], in_=poolacc[:, w * P:(w + 1) * P],
                                     func=AF.Exp)
                nc.scalar.activation(out=gts[:], in_=ge_[:], func=AF.Ln,
                                     bias=1.0, scale=1.0)
                g2ps = ppool.tile([P, HD], f32, space="PSUM", tag="ps")
                nc.tensor.matmul(out=g2ps[:], lhsT=gts[:], rhs=fcw_t[:],
                                 start=True, stop=True)
                g2b = dpool.tile([P, HD], f32, tag="g2b")
                nc.vector.tensor_tensor(out=g2b[:], in0=g2ps[:], in1=fcb_b[:], op=OP.add)
                g2e = dpool.tile([P, HD], f32, tag="g2e")
                nc.scalar.activation(out=g2e[:], in_=g2b[:], func=AF.Exp)
                g2 = dpool.tile([P, HD], f32, tag="g2")
                nc.scalar.activation(out=g2[:], in_=g2e[:], func=AF.Ln,
                                     bias=1.0, scale=1.0)
                g2tp = ppool.tile([P, P], f32, space="PSUM", tag="ps")
                nc.tensor.transpose(out=g2tp[:], in_=g2[:], identity=ident[:])
                g2ts = dpool.tile([P, P], f32, tag="g2ts")
                nc.vector.tensor_copy(out=g2ts[:], in_=g2tp[:])
                y_ps = ppool.tile([P, 1], f32, space="PSUM", tag="ps")
                nc.tensor.matmul(out=y_ps[:], lhsT=g2ts[:], rhs=fcow_t[:],
                                 start=True, stop=True)
                y_t = dpool.tile([P, 1], f32, tag="yt")
                nc.vector.tensor_tensor(out=y_t[:], in0=y_ps[:], in1=fcob_b[:],
                                        op=OP.add)
                nc.sync.dma_start(out=OUT[w * P:(w + 1) * P, :], in_=y_t[:])
    return nc


# ----------------------------------------------------------------------------
def _in_maps(pre, x, W_emb, b_emb, conv_W, bn_gamma, bn_beta,
             fc_W, fc_b, fco_W, fco_b):
    N = pre["N"]; NS = pre["NS"]
    AD = x.shape[1]
    convW2 = np.concatenate(
        [np.concatenate([w, w], axis=0) for w in np.asarray(conv_W, np.float32)],
        axis=0)
    common = dict(
        W_emb=np.asarray(W_emb, np.float32),
        b_emb=np.asarray(b_emb, np.float32).reshape(1, -1),
        convW2=convW2,
        bn_gc=np.asarray(bn_gamma, np.float32).T.copy(),
        bn_bc=np.asarray(bn_beta, np.float32).T.copy(),
        fc_W=np.asarray(fc_W, np.float32),
        fc_b=np.asarray(fc_b, np.float32).reshape(1, -1),
        fco_W=np.asarray(fco_W, np.float32),
        fco_b=np.asarray(fco_b, np.float32).reshape(1, 1),
        cinv=pre["cinv"], giota=pre["giota"],
    )
    maps = []
    for c in range(NCORES):
        lo = c * NS
        real = min(NS, max(0, N - lo))
        xs = np.zeros((AD, NS), np.float32)
        xs[:, :real] = np.asarray(x, np.float32)[lo:lo + real].T
        m = dict(common)
        m.update(
            xTs=xs,
            p1_idx=_wrap16(pre["idx1"][c]),
            p1_sc=pre["sc1"][c],
            p2_idx=_wrap16(pre["idx2"][c]),
            p2_sc=pre["sc2"][c],
            p2_sx=_wrap16(pre["sx2"][c]),
            btT=pre["bt"][c],
        )
        maps.append(m)
    return maps


def _run_pjrt_timed(nc, in_maps, n_cores, repeats=4):
    import time
    import jax
    from jax.experimental.shard_map import shard_map
    from jax.sharding import Mesh, PartitionSpec, NamedSharding
    from concourse import mybir
    from concourse.bass2jax import _bass_exec_p, partition_id_tensor, install_neuronx_cc_hook

    install_neuronx_cc_hook()
    in_names, out_names, out_avals, zero_outs = [], [], [], []
    partition_name = nc.partition_id_tensor.name if nc.partition_id_tensor else None
    for alloc in nc.m.functions[0].allocations:
        if not isinstance(alloc, mybir.MemoryLocationSet):
            continue
        name = alloc.memorylocations[0].name
        if alloc.kind == "ExternalInput":
            if name != partition_name:
                in_names.append(name)
        elif alloc.kind == "ExternalOutput":
            out_names.append(name)
            shape = tuple(alloc.tensor_shape)
            dtype = mybir.dt.np(alloc.dtype)
            out_avals.append(jax.core.ShapedArray(shape, dtype))
            zero_outs.append(np.zeros(shape, dtype))
    n_params = len(in_names)
    n_outs = len(out_avals)
    in_names.extend(out_names)
    if partition_name is not None:
        in_names.append(partition_name)
    donate = tuple(range(n_params, n_params + n_outs))

    def _body(*args):
        operands = list(args)
        if partition_name is not None:
            operands.append(partition_id_tensor())
        outs = _bass_exec_p.bind(
            *operands, out_avals=tuple(out_avals), in_names=tuple(in_names),
            out_names=tuple(out_names), lowering_input_output_aliases=(),
            sim_require_finite=True, sim_require_nnan=True, nc=nc)
        return tuple(outs)

    devices = jax.devices()[:n_cores]
    mesh = Mesh(np.asarray(devices), ("core",))
    in_specs = (PartitionSpec("core"),) * (n_params + n_outs)
    out_specs = (PartitionSpec("core"),) * len(out_names)
    sharded = jax.jit(
        shard_map(_body, mesh=mesh, in_specs=in_specs, out_specs=out_specs,
                  check_rep=False),
        donate_argnums=donate, keep_unused=True)
    sh = NamedSharding(mesh, PartitionSpec("core"))
    concat_in = [
        jax.device_put(
            np.concatenate([np.asarray(in_maps[c][nm]) for c in range(n_cores)],
                           axis=0), sh)
        for nm in in_names[:n_params]
    ]

    def zeros_dev():
        return [jax.device_put(np.zeros((n_cores * z.shape[0], *z.shape[1:]),
                                        z.dtype), sh) for z in zero_outs]

    out_arrs = sharded(*concat_in, *zeros_dev())
    jax.block_until_ready(out_arrs)
    results = [
        {nm: np.asarray(out_arrs[i]).reshape(n_cores, *out_avals[i].shape)[c]
         for i, nm in enumerate(out_names)}
        for c in range(n_cores)
    ]
    best_ns = None
    if bool(int(os.environ.get("KBENCH", "0"))):
        def timed_burst(k):
            zos = [zeros_dev() for _ in range(k)]
            for zo in zos:
                jax.block_until_ready(zo)
            t0 = time.perf_counter()
            outs = [sharded(*concat_in, *zo) for zo in zos]
            jax.block_until_ready(outs)
            return time.perf_counter() - t0
        best1 = min(timed_burst(1) for _ in range(repeats))
        bestk = min(timed_burst(4) for _ in range(max(2, repeats // 2)))
        per_exec = (bestk - best1) / 3.0
        print(f"[timing] single={best1*1e3:.2f} ms burst4={bestk*1e3:.2f} ms "
              f"-> per-exec={per_exec*1e6:.0f} us")
        best_ns = int(per_exec * 1e9)
    return results, best_ns


def kernel(x, W_emb, b_emb, conv_W, conv_b, bn_gamma, bn_beta,
           fc_W, fc_b, fco_W, fco_b, node_idx, edge_idx, batch, use_sim=False):
    x = np.asarray(x, np.float32)
    N, AD = x.shape
    batch = np.asarray(batch, np.int64)
    G = int(batch.max()) + 1
    pre = host_prep2(np.asarray(node_idx, np.int64),
                     np.asarray(edge_idx, np.int64), batch, N, G)
    NL = np.asarray(conv_W).shape[0]
    HD = np.asarray(fc_W).shape[1]
    nc = build_nc2(pre, AD, NL=NL, HD=HD)
    nc.finalize()
    maps = _in_maps(pre, x, W_emb, b_emb, conv_W, bn_gamma, bn_beta,
                    fc_W, fc_b, fco_W, fco_b)
    global LAST_EXEC_NS
    if use_sim:
        from concourse.bass_interp import MultiCoreSim
        sim = MultiCoreSim(nc, num_cores=NCORES, num_workers=8)
        for cid, cs in sim.cores.items():
            for name, arr in maps[cid].items():
                cs.tensor(name)[:] = arr
        sim.simulate()
        return np.array(sim.cores[0].tensor("OUT"))[:G].astype(np.float32)
    results, best_ns = _run_pjrt_timed(nc, maps, NCORES)
    LAST_EXEC_NS = best_ns
    return results[0]["OUT"][:G].astype(np.float32)
